# revision 47
# baseline (speedup 1.0000x reference)
"""Trainium2 Bass kernel for CISTransformerDecoder (deformable-attention decoder).

Sharding: 8 cores = 2 batches x 4 head-groups (2 heads each).
v2: quad-corner gather tables — each 256B table row p holds the 4 bilinear
corners [v(p), v(p+1), v(p+W), v(p+W+1)] so ONE dma_gather unit serves a
whole sample point (halves gather DMA + desc-gen vs 2-row scheme).
x0/y0 are clamped to >=0 and the bilinear weights permuted so every row
index is a valid flat position (no pad rows).  Tables for layer i+1 are
built inside layer i to overlap their DRAM writes with compute.
Per-core: all 1024 queries x 2 heads; 4-core ReduceScatter combines heads,
FFN runs on the local 256-query slice, AllGather restores src.
"""
import sys, math, contextlib
sys.path.insert(0, "/opt/trn_rl_repo")
import numpy as np

import concourse.bass as bass
import concourse.bacc as bacc
import concourse.tile as tile
from concourse import mybir
from concourse import library_config
from concourse.bass_utils import run_bass_kernel_spmd
from concourse.masks import make_identity

F32, BF16, I32, I16 = (mybir.dt.float32, mybir.dt.bfloat16, mybir.dt.int32,
                       mybir.dt.int16)
AF = mybir.ActivationFunctionType
AL = mybir.AluOpType
AX = mybir.AxisListType

SHAPES = ((128, 128), (64, 64), (32, 32), (16, 16))
NL, NH, NP, D, HD = 4, 8, 4, 256, 32
NLAYERS, DFF, BS, NQ = 6, 1024, 2, 1024
LEN = sum(h * w for h, w in SHAPES)        # 21760
NCORES, HPC = 8, 2
NQB = NQ // 128                             # 8
ELEM = 128                                  # bf16 elems per table row (256B)
LVL_W = [w for (h, w) in SHAPES]
LVL_H = [h for (h, w) in SHAPES]
LVL_BASE = [1, 1 + 16384, 1 + 16384 + 4096, 1 + 16384 + 4096 + 1024]
BLK = [128, 32, 8, 2]                       # positions-per-partition stride
WBS = [1, 2, 4, 8]                          # W // BLK (partition shift for +W)
TOFFE = [0, 129, 162, 171]                  # sts trow start (incl +1 ext row)
STR = 174                                   # sts trows (4 levels + ext rows)
TROWS_Q = LEN + 2                           # table rows; 1..21760 used
SMAX = float(LEN)                           # idx clip hi (rows 1..LEN)
# quad-build chunks: (level, row0-within-level, nrows)
QCH = [(0, 0, 32), (0, 32, 32), (0, 64, 32), (0, 96, 32), (1, 0, 32),
       (2, 0, 8), (3, 0, 2)]

_CACHE = {}
SKIP_GATHER = False
SKIP_CC = False


def _ap(t, off, dims):
    return bass.AP(t.tensor, t.offset + off, dims)


def _p0(t):
    return list(t.ap[0])


def _qmaj(dram_t, n):
    # DRAM AP for [(a p), n] laid out query-major, enumerated as (p, a, n)
    t = dram_t if isinstance(dram_t, bass.AP) else dram_t.ap()
    return bass.AP(t.tensor, t.offset, [[n, 128], [128 * n, NQB], [1, n]])


def build_nc(debug=False):
    nc = bacc.Bacc("TRN2", target_bir_lowering=False, debug=False,
                   num_devices=NCORES)
    dt = nc.dram_tensor
    ins = {}
    ins["tgts"] = dt("tgts", [NQ, D], F32, kind="ExternalInput")
    ins["refc"] = dt("refc", [4, NQ], F32, kind="ExternalInput")
    for l in range(NL):
        hw = LVL_H[l] * LVL_W[l]
        ins[f"mem{l}"] = dt(f"mem{l}", [D, hw], F32, kind="ExternalInput")
        ins[f"pm{l}"] = dt(f"pm{l}", [D, hw], F32, kind="ExternalInput")
    ins["lemb"] = dt("lemb", [NL, D], F32, kind="ExternalInput")
    for nm in ("ax", "ay"):
        ins[nm] = dt(nm, [NQ, 32], F32, kind="ExternalInput")
    for nm in ("bx", "by"):
        ins[nm] = dt(nm, [NQ, 32], BF16, kind="ExternalInput")
    ins["cw"] = dt("cw", [6, 32], F32, kind="ExternalInput")
    ins["ds"] = dt("ds", [128, 2], F32, kind="ExternalInput")
    ins["rp_w1"] = dt("rp_w1", [512, D], F32, kind="ExternalInput")
    ins["rp_b1"] = dt("rp_b1", [D], F32, kind="ExternalInput")
    ins["rp_w2"] = dt("rp_w2", [D, D], F32, kind="ExternalInput")
    ins["rp_b2"] = dt("rp_b2", [D], F32, kind="ExternalInput")
    ins["soaw_w"] = dt("soaw_w", [NLAYERS, D, 96], BF16,
                       kind="ExternalInput")
    ins["soaw_b"] = dt("soaw_b", [NLAYERS, 96], F32, kind="ExternalInput")
    ins["vp_w"] = dt("vp_w", [NLAYERS, D, HPC * HD], BF16, kind="ExternalInput")
    ins["opw_aug"] = dt("opw_aug", [NLAYERS, 68, D], BF16, kind="ExternalInput")
    ins["f1_w"] = dt("f1_w", [NLAYERS, D, DFF], BF16, kind="ExternalInput")
    ins["f1_b"] = dt("f1_b", [NLAYERS, DFF], F32, kind="ExternalInput")
    ins["f2_w"] = dt("f2_w", [NLAYERS, DFF, D], BF16, kind="ExternalInput")
    ins["f2_b"] = dt("f2_b", [NLAYERS, D], F32, kind="ExternalInput")
    for nm in ("n1_s", "n1_b", "n2_s", "n2_b"):
        ins[nm] = dt(nm, [NLAYERS, D], F32, kind="ExternalInput")

    out = dt("out", [NQ, D], F32, kind="ExternalOutput")

    groups = [[0, 1, 2, 3], [4, 5, 6, 7]]

    with tile.TileContext(nc) as tc:
        _build_body(nc, tc, ins, out, groups)
    nc.compile()
    return nc


def _build_body(nc, tc, ins, out, groups):
    ctx = contextlib.ExitStack()
    consts = ctx.enter_context(tc.tile_pool(name="consts", bufs=1))
    persist = ctx.enter_context(tc.tile_pool(name="persist", bufs=1))
    dpool = ctx.enter_context(tc.tile_pool(name="dpool", bufs=1, space="DRAM"))
    dwork = ctx.enter_context(tc.tile_pool(name="dwork", bufs=2, space="DRAM"))
    tables = [[dpool.tile([TROWS_Q, ELEM], BF16, tag=f"tbl_{i}_{h}",
                          name=f"tbl_{i}_{h}") for h in range(HPC)]
              for i in range(NLAYERS)]

    nc.gpsimd.load_library(library_config.mlp)

    ident = consts.tile([128, 128], F32)
    make_identity(nc, ident[:])
    identb = consts.tile([128, 144], BF16)
    nc.vector.memset(identb[:], 0.0)
    nc.scalar.activation(out=identb[:, 0:128], in_=ident[:], func=AF.Identity)

    # const tiles
    cw = consts.tile([128, 6, 32], F32)
    nc.sync.dma_start(out=cw[:], in_=bass.AP(ins["cw"].ap().tensor, 0,
                                             [[0, 128], [32, 6], [1, 32]]))
    locc = consts.tile([128, 2, NQB, 32], F32)
    loccb = consts.tile([128, 2, NQB, 32], BF16)
    for ci, nm in enumerate(("ax", "ay")):
        nc.sync.dma_start(out=locc[:, ci], in_=bass.AP(
            ins[nm].ap().tensor, 0, [[32, 128], [128 * 32, NQB], [1, 32]]))
    for ci, nm in enumerate(("bx", "by")):
        nc.sync.dma_start(out=loccb[:, ci], in_=bass.AP(
            ins[nm].ap().tensor, 0, [[32, 128], [128 * 32, NQB], [1, 32]]))
    lemb_t = consts.tile([128, NL, 2], F32)
    nc.sync.dma_start(out=lemb_t[:], in_=bass.AP(
        ins["lemb"].ap().tensor, 0, [[1, 128], [D, NL], [128, 2]]))

    # ================= phase 0: query_pos (transposed) =================
    qposT = [persist.tile([128, NQ], BF16, tag=f"qposT{m}",
                          name=f"qposT{m}") for m in range(2)]
    with tc.tile_pool(name="ph0", bufs=1) as ph0, \
         tc.tile_pool(name="ph0p", bufs=2, space="PSUM") as ph0p:
        ds = ph0.tile([128, 2], F32)
        nc.sync.dma_start(out=ds[:], in_=ins["ds"][:, :])
        qsT = [ph0.tile([128, NQ], F32, tag=f"qsT{c}", name=f"qsT{c}")
               for c in range(4)]
        for c in range(4):
            vrow = ph0.tile([128, NQ], F32, tag="vrow")
            nc.sync.dma_start(out=vrow[:], in_=bass.AP(
                ins["refc"].ap().tensor, c * NQ, [[0, 128], [1, NQ]]))
            sg = ph0.tile([128, NQ], F32, tag="sg")
            nc.scalar.activation(out=sg[:], in_=vrow[:], func=AF.Sigmoid)
            nc.scalar.activation(out=qsT[c][:], in_=sg[:], func=AF.Sin,
                                 scale=ds[:, 0:1], bias=ds[:, 1:2])
        rpb1 = ph0.tile([128, 2], F32)
        nc.sync.dma_start(out=rpb1[:], in_=bass.AP(
            ins["rp_b1"].ap().tensor, 0, [[1, 128], [128, 2]]))
        rpb2 = ph0.tile([128, 2], F32)
        nc.sync.dma_start(out=rpb2[:], in_=bass.AP(
            ins["rp_b2"].ap().tensor, 0, [[1, 128], [128, 2]]))
        w1t = ph0.tile([128, 4, D], F32)
        nc.sync.dma_start(out=w1t[:], in_=ins["rp_w1"][:, :].rearrange(
            "(c p) d -> p c d", p=128))
        w2t = ph0.tile([128, 2, D], F32)
        nc.sync.dma_start(out=w2t[:], in_=ins["rp_w2"][:, :].rearrange(
            "(c p) d -> p c d", p=128))
        h1T = [ph0.tile([128, NQ], F32, tag=f"h1T{m}", name=f"h1T{m}")
               for m in range(2)]
        for m in range(2):
            for qc in range(2):
                pt = ph0p.tile([128, 512], F32, tag="pmlp", name="pmlp")
                for c in range(4):
                    nc.tensor.matmul(out=pt[:],
                                     lhsT=w1t[:, c, m * 128:(m + 1) * 128],
                                     rhs=qsT[c][:, qc * 512:(qc + 1) * 512],
                                     start=(c == 0), stop=(c == 3))
                nc.scalar.activation(out=h1T[m][:, qc * 512:(qc + 1) * 512],
                                     in_=pt[:], func=AF.Relu,
                                     bias=rpb1[:, m:m + 1])
        for m in range(2):
            for qc in range(2):
                pt = ph0p.tile([128, 512], F32, tag="pmlp", name="pmlp")
                for c in range(2):
                    nc.tensor.matmul(out=pt[:],
                                     lhsT=w2t[:, c, m * 128:(m + 1) * 128],
                                     rhs=h1T[c][:, qc * 512:(qc + 1) * 512],
                                     start=(c == 0), stop=(c == 1))
                nc.vector.tensor_scalar(
                    out=qposT[m][:, qc * 512:(qc + 1) * 512], in0=pt[:],
                    scalar1=rpb2[:, m:m + 1], scalar2=None, op0=AL.add)

    # ================= phase 1: value_in =================
    vpw = consts.tile([128, NLAYERS, 2, HPC * HD], BF16)
    nc.sync.dma_start(out=vpw[:], in_=ins["vp_w"][:, :, :].rearrange(
        "l (c p) n -> p l c n", p=128))
    # build pools (persist across layers)
    bpool = ctx.enter_context(tc.tile_pool(name="bld", bufs=1))
    qpool = ctx.enter_context(tc.tile_pool(name="qdp", bufs=1))
    vpsum = ctx.enter_context(tc.tile_pool(name="vpsum", bufs=3, space="PSUM"))
    spsum = ctx.enter_context(tc.tile_pool(name="spsum", bufs=2, space="PSUM"))
    s2pool = ctx.enter_context(tc.tile_pool(name="s2p", bufs=1))
    sts = [bpool.tile([128, STR, HD], BF16, tag=f"sts{h}", name=f"sts{h}")
           for h in range(HPC)]
    for h in range(HPC):
        nc.vector.memset(sts[h][:], 0.0)

    vin = [persist.tile([128, LEN], BF16, tag=f"vin{c}", name=f"vin{c}")
           for c in range(2)]

    def _vin_level(l):
        hw = LVL_H[l] * LVL_W[l]
        base = LVL_BASE[l] - 1
        for c in range(2):
            for o in range(0, hw, 2048):
                wch = min(2048, hw - o)
                mt = vwork.tile([128, 2048], F32, tag="mt", name="mt")
                nc.sync.dma_start(out=mt[:, :wch],
                                  in_=ins[f"mem{l}"][c * 128:(c + 1) * 128,
                                                     o:o + wch])
                nc.gpsimd.dma_start(out=mt[:, :wch],
                                    in_=ins[f"pm{l}"][c * 128:(c + 1) * 128,
                                                      o:o + wch],
                                    accum_op=AL.add)
                nc.scalar.activation(out=vin[c][:, base + o:base + o + wch],
                                     in_=mt[:, :wch], func=AF.Identity,
                                     bias=lemb_t[:, l, c:c + 1])


    def _bt_vproj(li, levels=tuple(range(NL))):
        for l in levels:
            BL, base0, toff = BLK[l], LVL_BASE[l] - 1, TOFFE[l]
            for tb in range(0, BL, 8):
                nt = min(8, BL - tb)
                pt = vpsum.tile([128, 8, HPC * HD], F32, tag="vp", name="vp")
                for t in range(nt):
                    for c in range(2):
                        lhsT = _ap(vin[c], base0 + tb + t,
                                   [_p0(vin[c]), [BL, 128]])
                        nc.tensor.matmul(out=pt[:, t], lhsT=lhsT,
                                         rhs=vpw[:, li, c, :],
                                         start=(c == 0), stop=(c == 1))
                for h in range(HPC):
                    nc.scalar.activation(
                        out=sts[h][:, toff + tb:toff + tb + nt, :],
                        in_=_ap(pt, h * HD,
                                [_p0(pt), [HPC * HD, nt], [1, HD]]),
                        func=AF.Identity)
    def _bt_finish(li):
        for h in range(HPC):
            # ext rows: sts[k, toff+BL] = sts[k+1, toff]  (flat wrap)
            for l in range(NL):
                nc.sync.dma_start(out=sts[h][0:127, TOFFE[l] + BLK[l], :],
                                  in_=sts[h][1:128, TOFFE[l], :])
        tbls = tables[li]
        for h in range(HPC):
            # interleave into quad rows + write table; sts2 rows (the
            # +W partition shift) are produced per chunk via a column-
            # shifted-identity matmul.
            for (l, r0, nr) in QCH:
                BL, toff, base = BLK[l], TOFFE[l], LVL_BASE[l]
                wb = WBS[l]
                sts2 = s2pool.tile([128, 33, HD], BF16, tag="s2", name="s2")
                cols = (nr + 1) * HD
                co = 0
                while co < cols:
                    w = min(512, cols - co)
                    ps = spsum.tile([128, 512], F32, tag="ps2", name="ps2")
                    lhsT = _ap(identb, wb, [_p0(identb), [1, 128]])
                    rhs = _ap(sts[h], (toff + r0) * HD + co,
                              [_p0(sts[h]), [1, w]])
                    nc.tensor.matmul(out=ps[:, :w], lhsT=lhsT, rhs=rhs,
                                     start=True, stop=True)
                    nc.scalar.activation(
                        out=_ap(sts2, co, [_p0(sts2), [1, w]]),
                        in_=ps[:, :w], func=AF.Identity)
                    co += w
                qd = qpool.tile([128, 32, 4, HD], BF16, tag="qd", name="qd")
                for (cc, srct, dd) in ((0, sts[h], (toff + r0)),
                                       (1, sts[h], (toff + r0 + 1)),
                                       (2, sts2, 0), (3, sts2, 1)):
                    nc.vector.tensor_copy(
                        out=_ap(qd, cc * HD, [_p0(qd), [4 * HD, nr], [1, HD]]),
                        in_=_ap(srct, dd * HD,
                                [_p0(srct), [HD, nr], [1, HD]]))
                nc.sync.dma_start(
                    out=bass.AP(tbls[h].tensor, (base + r0) * ELEM,
                                [[BL * ELEM, 128], [ELEM, nr], [1, ELEM]]),
                    in_=qd[:, 0:nr])

    with tc.tile_pool(name="vwork", bufs=3) as vwork:
        for l in range(NL):
            _vin_level(l)
            _bt_vproj(0, (l,))

    # ================= main-loop pools + weights =================
    lwork = ctx.enter_context(tc.tile_pool(name="lwork", bufs=1))
    big = ctx.enter_context(tc.tile_pool(name="big", bufs=2))
    gpool = ctx.enter_context(tc.tile_pool(name="gp", bufs=3))
    ipool = ctx.enter_context(tc.tile_pool(name="ip", bufs=1))
    mpool = ctx.enter_context(tc.tile_pool(name="mp", bufs=2))
    psum = ctx.enter_context(tc.tile_pool(name="psum", bufs=2, space="PSUM"))
    psumb = ctx.enter_context(tc.tile_pool(name="psumb", bufs=1,
                                           space="PSUM"))
    f1bT = consts.tile([128, NLAYERS, 8], F32)
    nc.sync.dma_start(out=f1bT[:], in_=bass.AP(
        ins["f1_b"].ap().tensor, 0, [[1, 128], [DFF, NLAYERS], [128, 8]]))
    opw = consts.tile([68, NLAYERS, D], BF16)
    nc.sync.dma_start(out=opw[:], in_=ins["opw_aug"][:, :, :].rearrange(
        "l p n -> p l n"))
    fwins = (ins["f1_w"], ins["f2_w"])
    fbins = {nm: ins[nm] for nm in ("f2_b", "n1_s", "n1_b", "n2_s", "n2_b")}

    src = persist.tile([128, NQB, D], F32)
    nc.sync.dma_start(out=src[:], in_=ins["tgts"][:, :].rearrange(
        "(a p) d -> p a d", p=128))
    eps = consts.tile([128, 1], F32)
    nc.vector.memset(eps[:], 1e-5)

    args = dict(src=src, qposT=qposT, ins=ins, opw=opw, fwins=fwins,
                f1bT=f1bT, fbins=fbins, cw=cw, locc=locc, loccb=loccb,
                ident=ident, identb=identb, eps=eps, dwork=dwork,
                groups=groups, lwork=lwork, big=big, psum=psum,
                psumb=psumb, gpool=gpool, ipool=ipool, mpool=mpool)
    pre = _layer_pre(nc, 0, **args)
    _bt_finish(0)
    for li in range(NLAYERS):
        _layer_post(nc, li, pre, tables[li], (_bt_vproj, _bt_finish), **args)
        if li + 1 < NLAYERS:
            pre = _layer_pre(nc, li + 1, **args)

    nc.sync.dma_start(out=_qmaj(out, D), in_=src[:])
    ctx.close()


def _layer_pre(nc, li, src, qposT, ins, cw, locc, loccb,
               ident, lwork, big, psum, dwork, ipool, **_kw):
    P4 = [128, NQB, 2, 16]
    soawW = lwork.tile([128, 2, 96], BF16, tag="soawW", name="soawW")
    nc.sync.dma_start(out=soawW[:], in_=ins["soaw_w"][li, :, :].rearrange(
        "(c p) n -> p c n", p=128))
    soawB = lwork.tile([128, 96], F32, tag="soawB", name="soawB")
    nc.sync.dma_start(out=soawB[:], in_=bass.AP(
        ins["soaw_b"].ap().tensor, li * 96, [[0, 128], [1, 96]]))

    def T(tag, shape=None, dtp=F32):
        return lwork.tile(shape or P4, dtp, tag=tag, name=tag)

    def tt(o, i0, i1, op):
        nc.any.tensor_tensor(out=o, in0=i0, in1=i1, op=op)

    # ---- qT = srcT + qposT ----
    qT = [T(f"qT{m}", [128, NQ], BF16) for m in range(2)]
    for a in range(NQB):
        for m in range(2):
            pt = psum.tile([128, 512], F32, tag="pp", name="pp")
            nc.tensor.transpose(out=pt[:, 0:128],
                                in_=src[:, a, m * 128:(m + 1) * 128],
                                identity=ident[:])
            nc.scalar.activation(out=qT[m][:, a * 128:(a + 1) * 128],
                                 in_=pt[:, 0:128], func=AF.Copy)
    for m in range(2):
        tt(qT[m][:], qT[m][:], qposT[m][:], AL.add)

    # ---- so/aw ----
    soaw = T("soaw", [128, NQB, 96])
    for a in range(NQB):
        pt = psum.tile([128, 512], F32, tag="pp", name="pp")
        for m in range(2):
            nc.tensor.matmul(out=pt[:, 0:96],
                             lhsT=qT[m][:, a * 128:(a + 1) * 128],
                             rhs=soawW[:, m, :], start=(m == 0),
                             stop=(m == 1))
        nc.vector.tensor_tensor(out=soaw[:, a], in0=pt[:, 0:96], in1=soawB[:],
                                op=AL.add)

    # ---- softmax over 16 per (q, h) ----
    aw = T("aw", P4, BF16)
    mx = T("mx", [128, NQB, 2])
    awl = _ap(soaw, 64, [_p0(soaw), [96, NQB], [16, 2], [1, 16]])
    nc.vector.tensor_reduce(out=mx[:], in_=awl, axis=AX.X, op=AL.max)
    tt(aw[:], awl, _ap(mx, 0, [_p0(mx), [2, NQB], [1, 2], [0, 16]]),
       AL.subtract)
    nc.scalar.activation(out=aw[:], in_=aw[:], func=AF.Exp)
    sm = T("sm", [128, NQB, 2])
    nc.vector.tensor_reduce(out=sm[:], in_=aw[:], axis=AX.X, op=AL.add)
    nc.vector.reciprocal(out=sm[:], in_=sm[:])
    tt(aw[:], aw[:], _ap(sm, 0, [_p0(sm), [2, NQB], [1, 2], [0, 16]]), AL.mult)

    # ---- pixel coords ----
    sox = _ap(soaw, 0, [_p0(soaw), [96, NQB], [32, 2], [2, 16]])
    soy = _ap(soaw, 1, [_p0(soaw), [96, NQB], [32, 2], [2, 16]])

    def lc(ci):
        t = locc if ci in (0, 2) else loccb
        return _ap(t, (ci // 2) * NQB * 32,
                   [_p0(t), [32, NQB], [0, 2], [1, 16]])

    def cwv(r):
        return _ap(cw, r * 32, [_p0(cw), [0, NQB], [16, 2], [1, 16]])

    x, y = T("x"), T("y")
    tt(x[:], sox, lc(1), AL.mult)
    tt(x[:], x[:], lc(0), AL.add)
    tt(y[:], soy, lc(3), AL.mult)
    tt(y[:], y[:], lc(2), AL.add)

    ti = T("ti", P4, I16)
    tf = T("t1")
    fx, fy = T("fx"), T("fy")

    def floor_(dst, xin):
        nc.vector.tensor_copy(out=ti[:], in_=xin)
        nc.vector.tensor_copy(out=tf[:], in_=ti[:])
        nc.vector.tensor_tensor(out=dst[:], in0=tf[:], in1=xin, op=AL.is_gt)
        nc.vector.tensor_tensor(out=dst[:], in0=tf[:], in1=dst[:],
                                op=AL.subtract)

    floor_(fx, x[:])
    floor_(fy, y[:])
    lx, ly = T("lx", P4, BF16), T("ly", P4, BF16)
    tt(lx[:], x[:], fx[:], AL.subtract)
    tt(ly[:], y[:], fy[:], AL.subtract)

    t1 = T("t1")
    ix0, ix1 = T("iy0", P4, BF16), T("iy1", P4, BF16)
    iy0, iy1 = T("iy2", P4, BF16), T("iy3", P4, BF16)
    for (dst, f, hi) in ((ix0, fx, 3), (iy0, fy, 4)):
        nc.any.tensor_scalar(out=dst[:], in0=f[:], scalar1=0.0, scalar2=None,
                             op0=AL.is_ge)
        tt(t1[:], f[:], cwv(hi), AL.is_le)
        tt(dst[:], dst[:], t1[:], AL.mult)
    for (dst, f, hi) in ((ix1, fx, 5), (iy1, fy, -1)):
        nc.any.tensor_scalar(out=dst[:], in0=f[:], scalar1=-1.0, scalar2=None,
                             op0=AL.is_ge)
        if hi >= 0:
            tt(t1[:], f[:], cwv(hi), AL.is_le)          # fx <= W-2
        else:
            nc.any.tensor_scalar(out=t1[:], in0=f[:], scalar1=1.0,
                                 scalar2=None, op0=AL.add)
            tt(t1[:], t1[:], cwv(4), AL.is_le)          # fy+1 <= H-1
        tt(dst[:], dst[:], t1[:], AL.mult)

    wy0, wy1 = T("wy0", P4, BF16), T("wy1", P4, BF16)
    wx0, wx1 = T("ix0", P4, BF16), T("ix1", P4, BF16)
    nc.any.tensor_scalar(out=wy0[:], in0=ly[:], scalar1=-1.0, scalar2=-1.0,
                         op0=AL.add, op1=AL.mult)       # (ly-1)*-1
    tt(wy0[:], wy0[:], iy0[:], AL.mult)
    tt(wy0[:], wy0[:], aw[:], AL.mult)
    tt(wy1[:], ly[:], iy1[:], AL.mult)
    tt(wy1[:], wy1[:], aw[:], AL.mult)
    nc.any.tensor_scalar(out=wx0[:], in0=lx[:], scalar1=-1.0, scalar2=-1.0,
                         op0=AL.add, op1=AL.mult)
    tt(wx0[:], wx0[:], ix0[:], AL.mult)
    tt(wx1[:], lx[:], ix1[:], AL.mult)

    # clamp-and-permute: fold the x0/y0 < 0 case into the >=0 rows so every
    # gathered row is a valid flat position.  t_ = [f < 0] ->
    # wA = w0 + w1*t_ (w0 is 0 there via the i-mask), wB = w1 - w1*t_.
    t2 = T("ti", P4, BF16)
    for (f_, w0_, w1_) in ((fy, wy0, wy1), (fx, wx0, wx1)):
        nc.any.tensor_scalar(out=t1[:], in0=f_[:], scalar1=-1.0, scalar2=None,
                             op0=AL.is_le)
        tt(t2[:], w1_[:], t1[:], AL.mult)
        tt(w0_[:], w0_[:], t2[:], AL.add)
        tt(w1_[:], w1_[:], t2[:], AL.subtract)

    # wall2[q, a, h, u, c, 2dup] bf16 (pair-duplicated for DVE 2x mode)
    wall2 = T("wall2", [128, NQB, 2, 16, 4, 2], BF16)
    for c, (wy_, wx_) in enumerate(((wy0, wx0), (wy0, wx1),
                                    (wy1, wx0), (wy1, wx1))):
        nc.any.tensor_tensor(
            out=_ap(wall2, c * 2,
                    [_p0(wall2), [256, NQB], [128, 2], [8, 16], [1, 2]]),
            in0=_ap(wy_, 0, [_p0(wy_), [32, NQB], [16, 2], [1, 16], [0, 2]]),
            in1=_ap(wx_, 0, [_p0(wx_), [32, NQB], [16, 2], [1, 16], [0, 2]]),
            op=AL.mult)

    samp = T("samp", [128, NQB, 68], BF16)
    sumw = T("sumw", [128, NQB, 2])
    for h in range(HPC):
        nc.vector.tensor_reduce(
            out=_ap(sumw, h, [_p0(sumw), [2, NQB]]),
            in_=_ap(wall2, h * 128,
                    [_p0(wall2), [256, NQB], [8, 16], [2, 4]]),
            axis=AX.XY, op=AL.add)
    nc.scalar.activation(out=_ap(samp, 64, [_p0(samp), [68, NQB], [1, 2]]),
                         in_=sumw[:], func=AF.Identity)
    nc.vector.memset(samp[:, :, 66:68], 1.0)

    # ---- table row index: yb*W + xb + base, clip [1, LEN] ----
    s0 = T("x")  # reuse dead slot
    nc.any.tensor_scalar(out=tf[:], in0=fy[:], scalar1=0.0, scalar2=None,
                         op0=AL.max)
    tt(s0[:], tf[:], cwv(0), AL.mult)
    nc.any.tensor_scalar(out=tf[:], in0=fx[:], scalar1=0.0, scalar2=None,
                         op0=AL.max)
    tt(s0[:], s0[:], tf[:], AL.add)
    tt(s0[:], s0[:], cwv(2), AL.add)
    nc.any.tensor_scalar(out=s0[:], in0=s0[:], scalar1=1.0, scalar2=SMAX,
                         op0=AL.max, op1=AL.min)

    # ---- idx staging: [128q, (h,u)] -> idxT [32, NQ] i16 -> DRAM wrap ----
    idxT = T("qT0", [32, NQ], I16)
    for a in range(NQB):
        pt = psum.tile([128, 512], F32, tag="pp", name="pp")
        nc.tensor.transpose(out=pt[0:32, 0:128],
                            in_=_ap(s0, a * 32, [_p0(s0), [1, 32]]),
                            identity=ident[:])
        nc.vector.tensor_copy(out=idxT[:, a * 128:(a + 1) * 128],
                              in_=pt[0:32, 0:128])
    idx_dram = dwork.tile([16, 2 * NQ], I16, tag="idxd", name="idxd")
    pit = _p0(idxT)[0]
    for h in range(2):
        for a in range(NQB):
            nc.sync.dma_start(
                out=bass.AP(idx_dram.tensor,
                            idx_dram.offset + h * 1024 + a * 128,
                            [[8, 16], [1, 8], [2048, 16]]),
                in_=bass.AP(idxT.tensor,
                            idxT.offset + h * 16 * pit + a * 128,
                            [[pit, 16], [16, 8], [1, 16]]))
    # preload all idx tiles (double-buffered across layers) so the
    # gather loop's DMA slots are all 256B-gather transfers
    idx16all = ipool.tile([128, 2, NQB, 128], I16, tag="i16a",
                          name="i16a")
    for h in range(HPC):
        nc.sync.dma_start(out=idx16all[:, h], in_=bass.AP(
            idx_dram.tensor, idx_dram.offset + h * 1024,
            [[0, 8], [2048, 16], [1, 1024]]))
    return dict(wall2=wall2, samp=samp, idx16all=idx16all)


def _layer_post(nc, li, pre, tbls, build_tables, src, qposT, ins, opw,
                fwins, f1bT, fbins, cw, locc, ident, identb, eps, dwork,
                groups, lwork, big, psum, psumb, gpool, ipool, mpool,
                **_kw):
    wall2, samp, idx16all = pre["wall2"], pre["samp"], pre["idx16all"]
    f1w = lwork.tile([128, 2, DFF], BF16, tag="f1w", name="f1w")
    nc.sync.dma_start(out=f1w[:], in_=fwins[0][li, :, :].rearrange(
        "(c p) n -> p c n", p=128))
    f2w = lwork.tile([128, 8, D], BF16, tag="f2w", name="f2w")
    nc.sync.dma_start(out=f2w[:], in_=fwins[1][li, :, :].rearrange(
        "(c p) n -> p c n", p=128))

    def T(tag, shape=None, dtp=F32):
        return lwork.tile(shape or [128, NQB, 2, 16], dtp, tag=tag, name=tag)

    # next layer's value-projection: PE is idle during the gather window
    if li + 1 < NLAYERS:
        build_tables[0](li + 1)

    # ---- gather + weighted tree-reduce; attn streamed per a-block ----
    sampT = T("qT1", [68, NQ], BF16)
    cc_in = dwork.tile([NQ, D], F32, tag="cc_in", name="cc_in")
    for a in range(NQB):
        for h in range(HPC):
            g = gpool.tile([128, 16, ELEM], BF16, tag="g", name="g")
            if SKIP_GATHER:
                nc.vector.memset(g[:], 0.25)
            else:
                nc.gpsimd.dma_gather(
                    out_ap=g[:], in_ap=tbls[h][:, :],
                    idxs_ap=idx16all[:, h, a, :],
                    num_idxs=2048, num_idxs_reg=2048, elem_size=ELEM,
                    single_packet=False)
            # m = g * w — separate output tile releases g after one op
            # so the gather pipeline stays deep
            m = mpool.tile([128, 16, ELEM], BF16, tag="m", name="m")
            nc.vector.tensor_tensor(
                out=_ap(m, 0, [_p0(m), [128, 16], [32, 4], [2, 16], [1, 2]]),
                in0=_ap(g, 0, [_p0(g), [128, 16], [32, 4], [2, 16], [1, 2]]),
                in1=_ap(wall2, a * 256 + h * 128,
                        [_p0(wall2), [8, 16], [2, 4], [0, 16], [1, 2]]),
                op=AL.mult)
            # corner tree: (c0,c1) += (c2,c3); c0 += c1
            nc.vector.tensor_tensor(
                out=_ap(m, 0, [_p0(m), [128, 16], [1, 64]]),
                in0=_ap(m, 0, [_p0(m), [128, 16], [1, 64]]),
                in1=_ap(m, 64, [_p0(m), [128, 16], [1, 64]]), op=AL.add)
            nc.vector.tensor_tensor(
                out=_ap(m, 0, [_p0(m), [128, 16], [1, 32]]),
                in0=_ap(m, 0, [_p0(m), [128, 16], [1, 32]]),
                in1=_ap(m, 32, [_p0(m), [128, 16], [1, 32]]), op=AL.add)
            # unit tree over 16
            for k in (8, 4, 2):
                nc.vector.tensor_tensor(
                    out=_ap(m, 0, [_p0(m), [128, k], [1, 32]]),
                    in0=_ap(m, 0, [_p0(m), [128, k], [1, 32]]),
                    in1=_ap(m, k * 128, [_p0(m), [128, k], [1, 32]]),
                    op=AL.add)
            nc.vector.tensor_tensor(
                out=_ap(samp, a * 68 + h * 32, [_p0(samp), [1, 32]]),
                in0=_ap(m, 0, [_p0(m), [1, 32]]),
                in1=_ap(m, 128, [_p0(m), [1, 32]]),
                op=AL.add)
        # this a-block's attn path: transpose, op-project, fold src/4 and
        # stage for the ReduceScatter, all while later gathers stream
        ptb = psumb.tile([128, 128], BF16, tag="ptrb", name="ptrb")
        nc.tensor.transpose(out=ptb[:68, :], in_=samp[:, a, :],
                            identity=identb[:, 0:128])
        nc.scalar.activation(out=sampT[:, a * 128:(a + 1) * 128],
                             in_=ptb[:68, :], func=AF.Copy)
        pt = psum.tile([128, 512], F32, tag="pp", name="pp")
        nc.tensor.matmul(out=pt[:, 0:D], lhsT=sampT[:, a * 128:(a + 1) * 128],
                         rhs=opw[:, li, :], start=True, stop=True)
        attn_a = big.tile([128, D], F32, tag="attn_a", name="attn_a")
        nc.vector.scalar_tensor_tensor(out=attn_a[:], in0=src[:, a],
                                       scalar=0.25, in1=pt[:, 0:D],
                                       op0=AL.mult, op1=AL.add)
        nc.sync.dma_start(
            out=bass.AP(cc_in.tensor, cc_in.offset + a * 128 * D,
                        [[D, 128], [1, D]]),
            in_=attn_a[:])

    fb = {}
    for nm, dr in fbins.items():
        fb[nm] = lwork.tile([128, D], BF16, tag=f"fb_{nm}",
                            name=f"fb_{nm}")
        nc.gpsimd.dma_start(out=fb[nm][:], in_=bass.AP(
            dr.ap().tensor, li * D, [[0, 128], [1, D]]))

    NQL = NQB // 4  # local q-blocks after reduce-scatter

    def _qmaj2(dram_t, n):
        t = dram_t if isinstance(dram_t, bass.AP) else dram_t.ap()
        return bass.AP(t.tensor, t.offset, [[n, 128], [128 * n, NQL], [1, n]])

    rs_out = dwork.tile([NQ // 4, D], F32, tag="rs_out", name="rs_out")
    if not SKIP_CC:
        nc.gpsimd.collective_compute(
            "ReduceScatter", AL.add, replica_groups=groups,
            ins=[cc_in[:].opt()], outs=[rs_out[:].opt()])
    xs = T("xs", [128, NQL, D])
    nc.sync.dma_start(out=xs[:], in_=_qmaj2(rs_out, D))

    _layernorm(nc, None, xs, fb["n1_s"], fb["n1_b"], eps, lwork, xs, nblk=NQL)

    # ---- FFN on the local 256-query slice ----
    s1T = [T(f"s1T{m}", [128, 128 * NQL], BF16) for m in range(2)]
    for a in range(NQL):
        for m in range(2):
            pt = psum.tile([128, 512], F32, tag="pp", name="pp")
            nc.tensor.transpose(out=pt[:, 0:128],
                                in_=xs[:, a, m * 128:(m + 1) * 128],
                                identity=ident[:])
            nc.scalar.activation(out=s1T[m][:, a * 128:(a + 1) * 128],
                                 in_=pt[:, 0:128], func=AF.Copy)
    hT = [T(f"hT{m8}", [128, 128 * NQL], BF16) for m8 in range(8)]
    for m8 in range(8):
        pt = psum.tile([128, 512], F32, tag="pp", name="pp")
        for m in range(2):
            nc.tensor.matmul(out=pt[:, 0:128 * NQL],
                             lhsT=f1w[:, m, m8 * 128:(m8 + 1) * 128],
                             rhs=s1T[m][:], start=(m == 0), stop=(m == 1))
        nc.scalar.activation(out=hT[m8][:], in_=pt[:, 0:128 * NQL],
                             func=AF.Relu, bias=f1bT[:, li, m8:m8 + 1])
    ffn = T("ffn2", [128, NQL, D])
    for a in range(NQL):
        pt = psum.tile([128, 512], F32, tag="pp", name="pp")
        for m8 in range(8):
            nc.tensor.matmul(out=pt[:, 0:D],
                             lhsT=hT[m8][:, a * 128:(a + 1) * 128],
                             rhs=f2w[:, m8, :], start=(m8 == 0),
                             stop=(m8 == 7))
        nc.vector.tensor_tensor(out=ffn[:, a], in0=pt[:, 0:D],
                                in1=fb["f2_b"][:], op=AL.add)
    _layernorm(nc, xs, ffn, fb["n2_s"], fb["n2_b"], eps, lwork, xs, nblk=NQL)

    ag_in = dwork.tile([NQ // 4, D], F32, tag="ag_in", name="ag_in")
    ag_out = dwork.tile([NQ, D], F32, tag="ag_out", name="ag_out")
    nc.sync.dma_start(out=_qmaj2(ag_in, D), in_=xs[:])
    if not SKIP_CC:
        nc.gpsimd.collective_compute(
            "AllGather", AL.bypass, replica_groups=groups,
            ins=[ag_in[:].opt()], outs=[ag_out[:].opt()])
    nc.sync.dma_start(out=src[:], in_=_qmaj(ag_out, D))
    # finish stage (shifts + interleave + table writes) at the layer end:
    # its DVE copies overlap the AllGather -> qT-transpose wait of the next
    # layer's pre; vproj at the top of post already drained its deps.
    if li + 1 < NLAYERS:
        build_tables[1](li + 1)


def _layernorm(nc, src, delta, gamma_t, beta_t, eps, lwork, out_tile, nblk=NQB):
    # delta <- src + delta (skipped if src is None);
    # out_tile <- LN(delta)*gamma + beta
    if src is not None:
        nc.any.tensor_tensor(out=delta[:], in0=src[:], in1=delta[:], op=AL.add)
    stats = lwork.tile([128, nblk, 6], F32, tag="ln_st", name="ln_st")
    mv = lwork.tile([128, nblk, 2], F32, tag="ln_mv", name="ln_mv")
    for a in range(nblk):
        nc.vector.bn_stats(out=stats[:, a], in_=delta[:, a])
        nc.vector.bn_aggr(out=mv[:, a], in_=stats[:, a])
    rstd = lwork.tile([128, nblk], F32, tag="ln_rs", name="ln_rs")
    nc.scalar.activation(out=rstd[:], in_=_ap(mv, 1, [_p0(mv), [2, nblk]]),
                         func=AF.Sqrt, bias=eps[:, 0:1])
    nc.vector.reciprocal(out=rstd[:], in_=rstd[:])
    nmr = lwork.tile([128, nblk], F32, tag="ln_nm", name="ln_nm")
    nc.vector.tensor_tensor(out=nmr[:], in0=_ap(mv, 0, [_p0(mv), [2, nblk]]),
                            in1=rstd[:], op=AL.mult)
    nc.any.tensor_scalar(out=nmr[:], in0=nmr[:], scalar1=-1.0, scalar2=None,
                         op0=AL.mult)
    for a in range(nblk):
        nc.vector.tensor_scalar(out=delta[:, a], in0=delta[:, a],
                                scalar1=rstd[:, a:a + 1],
                                scalar2=nmr[:, a:a + 1],
                                op0=AL.mult, op1=AL.add)
    g = _ap(gamma_t, 0, [_p0(gamma_t), [0, nblk], [1, D]])
    b = _ap(beta_t, 0, [_p0(beta_t), [0, nblk], [1, D]])
    nc.any.tensor_tensor(out=delta[:], in0=delta[:], in1=g, op=AL.mult)
    nc.any.tensor_tensor(out=out_tile[:], in0=delta[:], in1=b, op=AL.add)


# ======================= host side =======================

def _expand32(v_ql):
    return np.ascontiguousarray(
        np.broadcast_to(v_ql[:, None, :, None], (NQ, HPC, NL, NP))
        .reshape(NQ, 32)).astype(np.float32)


def _host_inputs(inputs):
    import ml_dtypes
    tgts = np.asarray(inputs["tgts"], np.float32)
    refp = np.asarray(inputs["reference_points"], np.float32)
    masks = [np.asarray(inputs[f"mask{l}"]) for l in range(NL)]
    vrs = []
    for m in masks:
        H, W = m.shape[1], m.shape[2]
        vh = (~m[:, :, 0]).sum(1).astype(np.float32) / H
        vw = (~m[:, 0, :]).sum(1).astype(np.float32) / W
        vrs.append(np.stack([vw, vh], -1))
    vr = np.stack(vrs, 1)

    i = np.arange(128, dtype=np.float64)
    dim_t = 10000.0 ** (2 * np.floor(i / 2) / 128)
    ds = np.stack([2 * math.pi / dim_t,
                   np.where(np.arange(128) % 2 == 0, 0.0, math.pi / 2)],
                  -1).astype(np.float32)

    cw = np.zeros((6, 32), np.float32)
    for h2 in range(HPC):
        for l in range(NL):
            sl = slice(h2 * 16 + l * NP, h2 * 16 + (l + 1) * NP)
            cw[0, sl] = LVL_W[l]
            cw[1, sl] = LVL_H[l]
            cw[2, sl] = LVL_BASE[l]
            cw[3, sl] = LVL_W[l] - 1
            cw[4, sl] = LVL_H[l] - 1
            cw[5, sl] = LVL_W[l] - 2

    so_w = np.asarray(inputs["so_w"], np.float32)
    so_b = np.asarray(inputs["so_b"], np.float32)
    aw_w = np.asarray(inputs["aw_w"], np.float32)
    aw_b = np.asarray(inputs["aw_b"], np.float32)
    vp_w = np.asarray(inputs["vp_w"], np.float32)
    vp_b = np.asarray(inputs["vp_b"], np.float32)
    op_w = np.asarray(inputs["op_w"], np.float32)
    op_b = np.asarray(inputs["op_b"], np.float32)
    Wv = np.array(LVL_W, np.float32)[None]
    Hv = np.array(LVL_H, np.float32)[None]

    in_maps = []
    for core in range(NCORES):
        b, hg = core // 4, core % 4
        h0 = 2 * hg
        m = {}
        m["tgts"] = tgts[b]
        m["refc"] = np.ascontiguousarray(refp[b].T[[1, 0, 2, 3]])
        for l in range(NL):
            m[f"mem{l}"] = np.ascontiguousarray(
                np.asarray(inputs[f"mem{l}"], np.float32)[b].reshape(D, -1))
            m[f"pm{l}"] = np.ascontiguousarray(
                np.asarray(inputs[f"pm{l}"], np.float32)[b].reshape(D, -1))
        m["lemb"] = np.asarray(inputs["level_embed"], np.float32)
        ri = refp[b][:, None, :] * np.concatenate([vr[b], vr[b]], -1)[None]
        m["ax"] = _expand32(ri[:, :, 0] * Wv - 0.5)
        m["bx"] = _expand32(ri[:, :, 2] * Wv * (0.5 / NP)).astype(
            ml_dtypes.bfloat16)
        m["ay"] = _expand32(ri[:, :, 1] * Hv - 0.5)
        m["by"] = _expand32(ri[:, :, 3] * Hv * (0.5 / NP)).astype(
            ml_dtypes.bfloat16)
        m["cw"] = cw
        m["ds"] = ds
        m["rp_w1"] = np.asarray(inputs["rp_w1"], np.float32)
        m["rp_b1"] = np.asarray(inputs["rp_b1"], np.float32)
        m["rp_w2"] = np.asarray(inputs["rp_w2"], np.float32)
        m["rp_b2"] = np.asarray(inputs["rp_b2"], np.float32)
        so_r = so_w.reshape(NLAYERS, D, NH, NL, NP, 2)
        aw_r = aw_w.reshape(NLAYERS, D, NH, NL, NP)
        sob_r = so_b.reshape(NLAYERS, NH, NL, NP, 2)
        awb_r = aw_b.reshape(NLAYERS, NH, NL, NP)
        m["soaw_w"] = np.ascontiguousarray(np.concatenate([
            so_r[:, :, h0:h0 + 2].reshape(NLAYERS, D, 64),
            aw_r[:, :, h0:h0 + 2].reshape(NLAYERS, D, 32)],
            -1)).astype(ml_dtypes.bfloat16)
        m["soaw_b"] = np.ascontiguousarray(np.concatenate([
            sob_r[:, h0:h0 + 2].reshape(NLAYERS, 64),
            awb_r[:, h0:h0 + 2].reshape(NLAYERS, 32)], -1))
        m["vp_w"] = np.ascontiguousarray(
            vp_w[:, :, h0 * HD:(h0 + 2) * HD]).astype(ml_dtypes.bfloat16)
        opa = np.zeros((NLAYERS, 68, D), np.float32)
        for lii in range(NLAYERS):
            opa[lii, 0:64] = op_w[lii, h0 * HD:(h0 + 2) * HD]
            for hh in range(HPC):
                sl = slice((h0 + hh) * HD, (h0 + hh + 1) * HD)
                opa[lii, 64 + hh] = vp_b[lii, sl] @ op_w[lii, sl]
            opa[lii, 66] = op_b[lii] / 4.0
        m["opw_aug"] = opa.astype(ml_dtypes.bfloat16)
        m["f1_w"] = np.asarray(inputs["f1_w"], np.float32).astype(
            ml_dtypes.bfloat16)
        m["f2_w"] = np.asarray(inputs["f2_w"], np.float32).astype(
            ml_dtypes.bfloat16)
        m["f1_b"] = np.asarray(inputs["f1_b"], np.float32)
        m["f2_b"] = np.asarray(inputs["f2_b"], np.float32)
        for nm in ("n1_s", "n1_b", "n2_s", "n2_b"):
            m[nm] = np.asarray(inputs[nm], np.float32)
        in_maps.append(m)
    return in_maps


def kernel(**inputs):
    if "nc" not in _CACHE:
        _CACHE["nc"] = build_nc(debug=False)
    nc = _CACHE["nc"]
    in_maps = _host_inputs(inputs)
    res = run_bass_kernel_spmd(nc, in_maps, list(range(NCORES)))
    return np.stack([res.results[0]["out"],
                     res.results[4]["out"]]).astype(np.float32)
